# revision 1
# baseline (speedup 1.0000x reference)
"""GAT (2-layer graph attention network) on 8 Trainium2 NeuronCores.

Strategy (1D node partition, per sharding hint):
  - Each core owns R = N/8 rows (nodes) of the attention matrix.
  - Layer 1: every core computes the full Wh = X @ W1 (cheap, replicated),
    plus score projections s+ = X @ (W1 @ a_first), s- = X @ (W1 @ a_second).
    Scores e[j, i] = leaky_relu(s+_i + s-_j + maskbias) are built in a single
    fused custom DVE op per tile; exp on the scalar engine (batched); the
    masked-softmax numerator AND denominator come out of the aggregation
    matmuls (denominator via a ones-vector matmul on the same z stream).
  - Between layers: AllGather of each core's [R, 64+aux] payload
    (Wh2 = h_local @ W2 plus fused score projections + a ones column).
  - Layer 2: same fused-score pipeline; denominator rides as column 64 of
    the stationary operand (m=65 <= 128), so it is free.

Numerics: matmuls in bf16 (fp32 PSUM accumulate), softmax math exact up to
bf16 rounding; mask handled as additive -100 before leaky_relu: contribution
of masked entries is < 1e-8 relative (exact in effect).
"""

import math
from contextlib import ExitStack
from dataclasses import dataclass

import numpy as np
import ml_dtypes

import concourse.bass as bass
import concourse.mybir as mybir
import concourse.tile as tile
from concourse import bacc
from concourse.bass_utils import run_bass_kernel_spmd

BF16 = ml_dtypes.bfloat16
ALPHA = 0.2
MASKBIAS = -100.0

# --------------------------------------------------------------------------
# Custom fused DVE ops (registered into concourse.dve_ops at import time)
# --------------------------------------------------------------------------

import concourse.dve_ops as dve_ops
from concourse.dve_spec import (
    Spec, Src0, Src1, C0, Zero, lower, maxx, select, _has_src1,
)
from concourse.dve_uop import DveOpSpec


def _make_specs():
    # out = max(y, alpha*y), y = (in0 + s0) + in1
    #   in0 = s1 broadcast [P, R]; s0 = s2 per-partition [P, 1];
    #   in1 = additive mask bias {0, -100}; imm2 = alpha
    from concourse.dve_spec import C2
    _y = (Src0 + C0) + Src1

    def _score_ref(in0, in1, s0, s1, imm2):
        y = in0.astype(np.float32) + s0 + in1.astype(np.float32)
        return np.maximum(y, y * imm2)

    score = Spec(body=maxx(_y, _y * C2), reference=_score_ref)

    # out = in0 > 0 ? in0 : in1 - s0   (elu with in1 = exp(in0), s0 = 1.0)
    def _elu_ref(in0, in1, s0, s1, imm2):
        x = in0.astype(np.float32)
        return np.where(x > 0, x, in1.astype(np.float32) - s0)

    elu = Spec(body=select(Src0 > Zero, Src0, Src1 - C0), reference=_elu_ref)
    return score, elu


def _register(name, spec):
    if name in dve_ops._SUB_OPCODE_FOR_NAME:
        for op in dve_ops.OPS:
            if op.name == name:
                return op
    row = max(dve_ops._SUB_OPCODE_FOR_NAME.values()) + 1
    assert row < 0x20
    shas = {}
    for ver in ("v3", "v4"):
        uops = lower(spec, ver=ver)
        shas[ver] = DveOpSpec(
            name=name, opcode=row, uops=uops, rd1_en=_has_src1(spec)
        ).sha(ver)
    op = dve_ops.DveOp(name, spec, subdim=False, uops_sha=shas)
    dve_ops.OPS.append(op)
    dve_ops.CUSTOM_DVE_SPECS[name] = spec
    dve_ops._SUB_OPCODE_FOR_NAME[name] = row
    return op


_SCORE_SPEC, _ELU_SPEC = _make_specs()
SCORE_LRELU = _register("SCORE_LRELU_GAT", _SCORE_SPEC)
ELU_SEL = _register("ELU_SEL_GAT", _ELU_SPEC)


# --------------------------------------------------------------------------
# Kernel configuration
# --------------------------------------------------------------------------

@dataclass(frozen=True)
class Cfg:
    N: int = 4096      # nodes
    C: int = 512       # input feature dim
    H: int = 128       # hidden per head (must be 128)
    HEADS: int = 4
    F2: int = 64       # output dim
    CORES: int = 8
    GRP: int = 4       # j-tiles per batched exp

    @property
    def R(self): return self.N // self.CORES          # rows per core
    @property
    def JT(self): return self.N // 128                # j tiles
    @property
    def CT(self): return self.C // 128                # input-feature tiles
    @property
    def HH(self): return self.HEADS * self.H          # layer-1 out features
    @property
    def CT2(self): return self.HH // 128              # layer-2 contraction tiles
    @property
    def RT(self): return self.R // 128                # local row tiles
    @property
    def S8(self): return 2 * self.HEADS               # score projections per node
    @property
    def PAY(self): return self.F2 + 4                 # gather payload cols (64|1|s1|s2|pad)
    @property
    def NG(self): return self.JT // self.GRP


FULL = Cfg()


# --------------------------------------------------------------------------
# Device program
# --------------------------------------------------------------------------

def build_gat_nc(cfg: Cfg, collective: bool = True, iters: int = 1,
                 score_plain: bool = False, elu_plain: bool = False,
                 bcast_pe: bool = False, loop_iters: int = 0,
                 phases: str = "full", gather_wh: bool = True,
                 act_lrelu: int = 0):
    dt = mybir.dt.bfloat16
    f32 = mybir.dt.float32
    add = mybir.AluOpType.add
    mult = mybir.AluOpType.mult
    bypass = mybir.AluOpType.bypass
    Exp = mybir.ActivationFunctionType.Exp

    N, C, HEADS, F2, R = cfg.N, cfg.C, cfg.HEADS, cfg.F2, cfg.R
    JT, CT, HH, CT2, RT = cfg.JT, cfg.CT, cfg.HH, cfg.CT2, cfg.RT
    S8, PAY, GRP, NG = cfg.S8, cfg.PAY, cfg.GRP, cfg.NG
    F2p = F2 + 2

    nc = bacc.Bacc(
        "TRN2", target_bir_lowering=False, debug=False, num_devices=cfg.CORES
    )

    # ---- DRAM I/O -------------------------------------------------------
    xt_d = nc.dram_tensor("xt", [128, CT * N], dt, kind="ExternalInput").ap()
    xtl_d = nc.dram_tensor("xtloc", [128, CT * R], dt, kind="ExternalInput").ap()
    mb_d = nc.dram_tensor("mb", [128, JT * R], dt, kind="ExternalInput").ap()
    w1c_d = nc.dram_tensor("w1c", [128, CT * HH], dt, kind="ExternalInput").ap()
    w1t_d = nc.dram_tensor("w1t", [128, HEADS * C], dt, kind="ExternalInput").ap()
    a1p_d = nc.dram_tensor("a1p", [128, HEADS * 2], dt, kind="ExternalInput").ap()
    w2_d = nc.dram_tensor("w2", [128, CT2 * F2p], dt, kind="ExternalInput").ap()
    w2t_d = nc.dram_tensor("w2t", [F2, HH], dt, kind="ExternalInput").ap()
    a2p_d = nc.dram_tensor("a2p", [F2, 2], dt, kind="ExternalInput").ap()
    id_d = nc.dram_tensor("ident", [128, 128], dt, kind="ExternalInput").ap()
    idf_d = nc.dram_tensor("identf", [128, 128], f32, kind="ExternalInput").ap()
    out_d = nc.dram_tensor("out", [R, F2], f32, kind="ExternalOutput").ap()

    with tile.TileContext(nc) as tc, ExitStack() as ctx:
        const = ctx.enter_context(tc.tile_pool(name="const", bufs=1))
        work = ctx.enter_context(tc.tile_pool(name="work", bufs=3))
        wz = ctx.enter_context(tc.tile_pool(name="wz", bufs=3))
        psb = ctx.enter_context(tc.tile_pool(name="psb", bufs=3, space="PSUM"))
        pss = ctx.enter_context(tc.tile_pool(name="pss", bufs=2, space="PSUM"))
        psd = ctx.enter_context(tc.tile_pool(name="psd", bufs=2, space="PSUM"))
        ps2 = ctx.enter_context(tc.tile_pool(name="ps2", bufs=1, space="PSUM"))
        dram = ctx.enter_context(tc.tile_pool(name="dram", bufs=1, space="DRAM"))

        gsend_t = dram.tile([128, RT * PAY], dt)
        if cfg.CORES > 4:
            gfull_t = nc.dram_tensor(
                "gfull_sh", [cfg.CORES * 128, RT * PAY], dt,
                addr_space="Shared").ap()
        else:
            gfull_t = dram.tile([cfg.CORES * 128, RT * PAY], dt)
        whsend_t = dram.tile([128, RT * HH], dt)
        if cfg.CORES > 4:
            whfull_t = nc.dram_tensor(
                "whfull_sh", [cfg.CORES * 128, RT * HH], dt,
                addr_space="Shared").ap()
        else:
            whfull_t = dram.tile([cfg.CORES * 128, RT * HH], dt)

        import contextlib
        loop_cm = (tc.For_i(0, loop_iters, 1) if loop_iters
                   else contextlib.nullcontext())
        with loop_cm:
          for _it in range(iters):
            # ---- constant loads --------------------------------------------
            xt_sb = const.tile([128, CT * N], dt)
            _hx = CT * N // 2
            nc.sync.dma_start(out=xt_sb[:, 0:_hx], in_=xt_d[:, 0:_hx])
            nc.sync.dma_start(out=xt_sb[:, _hx:], in_=xt_d[:, _hx:])
            xtl_sb = const.tile([128, CT * R], dt)
            nc.gpsimd.dma_start(out=xtl_sb, in_=xtl_d)
            mb_sb = const.tile([128, JT * R], dt)
            _hm = JT * R // 4
            for _q in range(4):
                nc.gpsimd.dma_start(
                    out=mb_sb[:, _q * _hm: (_q + 1) * _hm],
                    in_=mb_d[:, _q * _hm: (_q + 1) * _hm])
            w1c_sb = const.tile([128, CT * HH], dt)
            nc.sync.dma_start(out=w1c_sb, in_=w1c_d)
            w1t_sb = const.tile([128, HEADS * C], dt)
            nc.sync.dma_start(out=w1t_sb, in_=w1t_d)
            a1p_sb = const.tile([128, HEADS * 2], dt)
            nc.sync.dma_start(out=a1p_sb, in_=a1p_d)
            w2a_sb = const.tile([128, CT2 * F2p], dt)
            nc.sync.dma_start(out=w2a_sb, in_=w2_d)
            w2t_sb = const.tile([F2, HH], dt)
            nc.sync.dma_start(out=w2t_sb, in_=w2t_d)
            a2p_sb = const.tile([F2, 2], dt)
            nc.sync.dma_start(out=a2p_sb, in_=a2p_d)
            ident_sb = const.tile([128, 128], dt)
            nc.sync.dma_start(out=ident_sb, in_=id_d)
            identf_sb = const.tile([128, 128], f32)
            nc.sync.dma_start(out=identf_sb, in_=idf_d)

            ones_col = const.tile([128, 1], dt)
            nc.vector.memset(ones_col, 1.0)
            ones_row = const.tile([1, 128], dt)
            nc.vector.memset(ones_row, 1.0)
            onesf_row = const.tile([1, 128], f32)
            nc.vector.memset(onesf_row, 1.0)

            # ---- fused score-projection weights: w~ = W @ a_half -----------
            ws1_sb = const.tile([128, CT * S8], dt)  # [c%128, ct*S8 + 2h+half]
            for h in range(HEADS):
                for ct in range(CT):
                    pw = pss.tile([128, 2], f32, tag="sm")
                    nc.tensor.matmul(
                        out=pw,
                        lhsT=w1t_sb[:, h * C + ct * 128: h * C + (ct + 1) * 128],
                        rhs=a1p_sb[:, h * 2: h * 2 + 2],
                        start=True, stop=True,
                    )
                    nc.vector.tensor_copy(
                        out=ws1_sb[:, ct * S8 + 2 * h: ct * S8 + 2 * h + 2], in_=pw
                    )
            for ct in range(CT2):
                pw = pss.tile([128, 2], f32, tag="sm")
                nc.tensor.matmul(
                    out=pw,
                    lhsT=w2t_sb[0:F2, ct * 128: (ct + 1) * 128],
                    rhs=a2p_sb[0:F2, :],
                    start=True, stop=True,
                )
                nc.vector.tensor_copy(
                    out=w2a_sb[:, ct * F2p + F2: ct * F2p + F2 + 2], in_=pw
                )

            if phases == "dma":
                for rt in range(RT):
                    nc.sync.dma_start(out=out_d[rt * 128:(rt + 1) * 128, :],
                                      in_=identf_sb[:, 0:F2])
                continue
            # ---- layer-1 Wh (all heads) + per-node score projections -------
            wh_sb = const.tile([128, JT * HH], dt)   # Wh[j, :] tiles
            ssb = const.tile([128, JT * S8], f32)    # s projections per j
            for t in range(JT):
                pS = pss.tile([128, S8], f32, tag="sm")
                for ct in range(CT):
                    xsl = xt_sb[:, ct * N + t * 128: ct * N + (t + 1) * 128]
                    nc.tensor.matmul(
                        out=pS, lhsT=xsl, rhs=ws1_sb[:, ct * S8: (ct + 1) * S8],
                        start=(ct == 0), stop=(ct == CT - 1),
                    )
                nc.vector.tensor_copy(out=ssb[:, t * S8: (t + 1) * S8], in_=pS)
                if not gather_wh:
                    pA = psb.tile([128, HH], f32, tag="big")
                    for ct in range(CT):
                        xsl = xt_sb[:, ct * N + t * 128: ct * N + (t + 1) * 128]
                        nc.tensor.matmul(
                            out=pA, lhsT=xsl,
                            rhs=w1c_sb[:, ct * HH: (ct + 1) * HH],
                            start=(ct == 0), stop=(ct == CT - 1),
                        )
                    eng = nc.vector if (t % 2 == 0) else nc.scalar
                    if eng is nc.vector:
                        eng.tensor_copy(out=wh_sb[:, t * HH: (t + 1) * HH], in_=pA)
                    else:
                        eng.copy(out=wh_sb[:, t * HH: (t + 1) * HH], in_=pA)

            if gather_wh:
                # local Wh rows + AllGather, instead of replicated compute
                for rt in range(RT):
                    pA = psb.tile([128, HH], f32, tag="big")
                    for ct in range(CT):
                        nc.tensor.matmul(
                            out=pA,
                            lhsT=xtl_sb[:, ct * R + rt * 128: ct * R + (rt + 1) * 128],
                            rhs=w1c_sb[:, ct * HH: (ct + 1) * HH],
                            start=(ct == 0), stop=(ct == CT - 1),
                        )
                    whl = work.tile([128, HH], dt, tag="whl")
                    nc.vector.tensor_copy(out=whl, in_=pA)
                    nc.sync.dma_start(
                        out=whsend_t[:, rt * HH: (rt + 1) * HH], in_=whl)
                if collective:
                    nc.gpsimd.collective_compute(
                        "AllGather", bypass,
                        replica_groups=[list(range(cfg.CORES))],
                        ins=[whsend_t.opt()], outs=[whfull_t.opt()],
                    )
                else:
                    for c in range(cfg.CORES):
                        nc.sync.dma_start(
                            out=whfull_t[c * 128: (c + 1) * 128, :],
                            in_=whsend_t[:, :])
                for c in range(cfg.CORES):
                    nc.sync.dma_start(
                        out=wh_sb[:, c * RT * HH: (c + 1) * RT * HH],
                        in_=whfull_t[c * 128: (c + 1) * 128, :])
            if phases == "wh":
                for rt in range(RT):
                    nc.sync.dma_start(out=out_d[rt * 128:(rt + 1) * 128, :],
                                      in_=identf_sb[:, 0:F2])
                continue
            # ---- layer 1: per-head attention + aggregation ------------------
            hloc_sb = const.tile([128, CT2 * R], dt)  # h_local^T, feature-major
            s1r_sb = const.tile([1, HEADS * R], dt)
            for h in range(HEADS):
                ps1 = psd.tile([1, R], f32, tag="den")
                for ct in range(CT):
                    nc.tensor.matmul(
                        out=ps1,
                        lhsT=ws1_sb[:, ct * S8 + 2 * h: ct * S8 + 2 * h + 1],
                        rhs=xtl_sb[:, ct * R: (ct + 1) * R],
                        start=(ct == 0), stop=(ct == CT - 1),
                    )
                nc.vector.tensor_copy(out=s1r_sb[0:1, h * R: (h + 1) * R], in_=ps1)
                s1b = work.tile([128, R], dt, tag="s1b")
                if bcast_pe:
                    psb1 = psb.tile([128, R], f32, tag="big")
                    nc.tensor.matmul(out=psb1, lhsT=ones_row,
                                     rhs=s1r_sb[0:1, h * R: (h + 1) * R],
                                     start=True, stop=True)
                    nc.scalar.copy(out=s1b[:, :], in_=psb1)
                else:
                    nc.gpsimd.partition_broadcast(
                        out_ap=s1b[:, :], in_ap=s1r_sb[0:1, h * R: (h + 1) * R]
                    )

                psum_h = psb.tile([128, R], f32, tag="big")
                psum_d = psd.tile([1, R], f32, tag="den")
                for g in range(NG):
                    ug = wz.tile([128, GRP * R], dt, tag="ug")
                    zg = wz.tile([128, GRP * R], dt, tag="zg")
                    for k in range(GRP):
                        t = g * GRP + k
                        if score_plain:
                            nc.vector.tensor_tensor(
                                out=ug[:, k * R: (k + 1) * R], in0=s1b[:, :],
                                in1=mb_sb[:, t * R: (t + 1) * R], op=add)
                        elif k < act_lrelu:
                            u0 = work.tile([128, R], dt, tag="u0")
                            nc.vector.tensor_tensor(
                                out=u0, in0=s1b[:, :],
                                in1=mb_sb[:, t * R: (t + 1) * R], op=add)
                            nc.scalar.activation(
                                out=ug[:, k * R: (k + 1) * R], in_=u0,
                                func=mybir.ActivationFunctionType.Lrelu,
                                bias=ssb[:, t * S8 + 2 * h + 1: t * S8 + 2 * h + 2],
                                scale=1.0, alpha=ALPHA,
                            )
                        else:
                            nc.vector._custom_dve(
                                SCORE_LRELU,
                                out=ug[:, k * R: (k + 1) * R],
                                in0=s1b[:, :],
                                in1=mb_sb[:, t * R: (t + 1) * R],
                                s0=ssb[:, t * S8 + 2 * h + 1: t * S8 + 2 * h + 2],
                                s1=0.0,
                                imm2=ALPHA,
                            )
                    nc.scalar.activation(out=zg[:, :], in_=ug[:, :], func=Exp)
                    for k in range(GRP):
                        t = g * GRP + k
                        zt = zg[:, k * R: (k + 1) * R]
                        nc.tensor.matmul(
                            out=psum_h,
                            lhsT=wh_sb[:, t * HH + h * 128: t * HH + (h + 1) * 128],
                            rhs=zt,
                            start=(t == 0), stop=(t == JT - 1),
                        )
                        nc.tensor.matmul(
                            out=psum_d, lhsT=ones_col, rhs=zt,
                            start=(t == 0), stop=(t == JT - 1),
                        )
                # normalize + elu -> h_local^T tile for this head
                rcp = work.tile([1, R], f32, tag="rcp")
                nc.vector.reciprocal(out=rcp, in_=psum_d[0:1, :])
                prb = psb.tile([128, R], f32, tag="big")
                nc.tensor.matmul(
                    out=prb, lhsT=onesf_row, rhs=rcp[0:1, :], start=True, stop=True
                )
                rb_sb = work.tile([128, R], f32, tag="rb")
                nc.scalar.copy(out=rb_sb, in_=prb)
                hn = work.tile([128, R], f32, tag="hn")
                nc.vector.tensor_tensor(out=hn, in0=psum_h, in1=rb_sb, op=mult)
                eh = work.tile([128, R], dt, tag="eh")
                nc.scalar.activation(out=eh, in_=hn, func=Exp)
                if elu_plain:
                    nc.vector.tensor_copy(
                        out=hloc_sb[:, h * R: (h + 1) * R], in_=hn)
                else:
                    nc.vector._custom_dve(
                        ELU_SEL,
                        out=hloc_sb[:, h * R: (h + 1) * R],
                        in0=hn, in1=eh, s0=1.0, s1=0.0, imm2=0.0,
                    )

            if phases == "l1":
                for rt in range(RT):
                    nc.sync.dma_start(out=out_d[rt * 128:(rt + 1) * 128, :],
                                      in_=identf_sb[:, 0:F2])
                continue
            # ---- layer-2 local projections + gather payload -----------------
            gs_sb = const.tile([128, RT * PAY], dt)
            for rt in range(RT):
                pW = pss.tile([128, F2p], f32, tag="sm")
                for ct in range(CT2):
                    nc.tensor.matmul(
                        out=pW,
                        lhsT=hloc_sb[:, ct * R + rt * 128: ct * R + (rt + 1) * 128],
                        rhs=w2a_sb[:, ct * F2p: (ct + 1) * F2p],
                        start=(ct == 0), stop=(ct == CT2 - 1),
                    )
                b = rt * PAY
                nc.vector.tensor_copy(out=gs_sb[:, b: b + F2], in_=pW[:, 0:F2])
                nc.vector.memset(gs_sb[:, b + F2: b + F2 + 1], 1.0)
                nc.vector.tensor_copy(
                    out=gs_sb[:, b + F2 + 1: b + F2 + 3], in_=pW[:, F2: F2 + 2]
                )
                nc.vector.memset(gs_sb[:, b + F2 + 3: b + PAY], 0.0)
                nc.sync.dma_start(
                    out=gsend_t[:, b: b + PAY], in_=gs_sb[:, b: b + PAY],
                )

            if collective:
                nc.gpsimd.collective_compute(
                    "AllGather",
                    bypass,
                    replica_groups=[list(range(cfg.CORES))],
                    ins=[gsend_t.opt()],
                    outs=[gfull_t.opt()],
                )
            else:
                # timing-only variant (TimelineSim can't model collectives):
                # approximate the gather with DMAs of the same total volume
                for c in range(cfg.CORES):
                    nc.sync.dma_start(
                        out=gfull_t[c * 128: (c + 1) * 128, :],
                        in_=gsend_t[:, :]
                    )

            # ---- layer-2 prep ----------------------------------------------
            gf_sb = const.tile([128, JT * PAY], dt)
            for c in range(cfg.CORES):
                nc.sync.dma_start(
                    out=gf_sb[:, c * RT * PAY: (c + 1) * RT * PAY],
                    in_=gfull_t[c * 128: (c + 1) * 128, :],
                )
            s2pf = const.tile([128, JT], f32)
            nc.vector.tensor_copy(
                out=s2pf[:, :].rearrange("p (t o) -> p t o", o=1),
                in_=gf_sb[:, :].rearrange("p (t q) -> p t q", q=PAY)[
                    :, :, F2 + 2: F2 + 3
                ],
            )
            s1r2_sb = const.tile([1, R], dt)
            for rt in range(RT):
                pt12 = pss.tile([2, 128], dt, tag="sm")
                nc.tensor.transpose(
                    out=pt12,
                    in_=gs_sb[:, rt * PAY + F2 + 1: rt * PAY + F2 + 3],
                    identity=ident_sb,
                )
                nc.vector.tensor_copy(
                    out=s1r2_sb[0:1, rt * 128: (rt + 1) * 128], in_=pt12[0:1, :]
                )
            s1b2 = const.tile([128, R], dt)
            if bcast_pe:
                psb2 = psb.tile([128, R], f32, tag="big")
                nc.tensor.matmul(out=psb2, lhsT=ones_row, rhs=s1r2_sb[0:1, :],
                                 start=True, stop=True)
                nc.scalar.copy(out=s1b2[:, :], in_=psb2)
            else:
                nc.gpsimd.partition_broadcast(out_ap=s1b2[:, :], in_ap=s1r2_sb[0:1, :])

            # ---- layer-2 attention + aggregation ----------------------------
            psum2 = ps2.tile([F2 + 1, R], f32)
            for g in range(NG):
                ug = wz.tile([128, GRP * R], dt, tag="ug")
                zg = wz.tile([128, GRP * R], dt, tag="zg")
                for k in range(GRP):
                    t = g * GRP + k
                    if score_plain:
                        nc.vector.tensor_tensor(
                            out=ug[:, k * R: (k + 1) * R], in0=s1b2[:, :],
                            in1=mb_sb[:, t * R: (t + 1) * R], op=add)
                    elif k < act_lrelu:
                        u0 = work.tile([128, R], dt, tag="u0")
                        nc.vector.tensor_tensor(
                            out=u0, in0=s1b2[:, :],
                            in1=mb_sb[:, t * R: (t + 1) * R], op=add)
                        nc.scalar.activation(
                            out=ug[:, k * R: (k + 1) * R], in_=u0,
                            func=mybir.ActivationFunctionType.Lrelu,
                            bias=s2pf[:, t: t + 1],
                            scale=1.0, alpha=ALPHA,
                        )
                    else:
                        nc.vector._custom_dve(
                            SCORE_LRELU,
                            out=ug[:, k * R: (k + 1) * R],
                            in0=s1b2[:, :],
                            in1=mb_sb[:, t * R: (t + 1) * R],
                            s0=s2pf[:, t: t + 1],
                            s1=0.0,
                            imm2=ALPHA,
                        )
                nc.scalar.activation(out=zg[:, :], in_=ug[:, :], func=Exp)
                for k in range(GRP):
                    t = g * GRP + k
                    nc.tensor.matmul(
                        out=psum2,
                        lhsT=gf_sb[:, t * PAY: t * PAY + F2 + 1],
                        rhs=zg[:, k * R: (k + 1) * R],
                        start=(t == 0), stop=(t == JT - 1),
                    )

            # ---- finalize: transpose, normalize, store ----------------------
            o2 = const.tile([F2 + 1, R], f32)
            nc.vector.tensor_copy(out=o2, in_=psum2)
            for rt in range(RT):
                pT2 = pss.tile([128, F2 + 1], f32, tag="sm")
                nc.tensor.transpose(
                    out=pT2,
                    in_=o2[:, rt * 128: (rt + 1) * 128],
                    identity=identf_sb[0: F2 + 1, 0: F2 + 1],
                )
                rc = work.tile([128, 1], f32, tag="rc")
                nc.vector.reciprocal(out=rc, in_=pT2[:, F2: F2 + 1])
                of = work.tile([128, F2], f32, tag="of")
                nc.vector.tensor_scalar(
                    out=of, in0=pT2[:, 0:F2], scalar1=rc, scalar2=0.0,
                    op0=mult, op1=bypass,
                )
                nc.sync.dma_start(
                    out=out_d[rt * 128: (rt + 1) * 128, :], in_=of
                )

    nc.compile()
    return nc


# --------------------------------------------------------------------------
# Host-side prep / sharding
# --------------------------------------------------------------------------

def host_prep(cfg: Cfg, g, inputs, W1, a1, W2, a2):
    N, C, H, HEADS, F2, R = cfg.N, cfg.C, cfg.H, cfg.HEADS, cfg.F2, cfg.R
    X = np.asarray(inputs, np.float32)
    W1 = np.asarray(W1, np.float32)
    a1 = np.asarray(a1, np.float32)
    W2 = np.asarray(W2, np.float32)
    a2 = np.asarray(a2, np.float32)

    def tile128(A):
        # [k*128, cols] row-major -> partition-major [128, k*cols]
        k = A.shape[0] // 128
        return np.ascontiguousarray(
            A.reshape(k, 128, A.shape[1]).transpose(1, 0, 2).reshape(128, -1)
        )

    XT = np.ascontiguousarray(X.T).astype(BF16)                       # [C, N]
    xt_t = tile128(XT)
    w1c = tile128(np.ascontiguousarray(
        W1.transpose(1, 0, 2).reshape(C, HEADS * H)).astype(BF16))
    w1t = tile128(np.ascontiguousarray(
        W1.transpose(0, 2, 1).reshape(HEADS * H, C)).astype(BF16))
    a1p = tile128(np.ascontiguousarray(
        np.stack([a1[:, :H, 0], a1[:, H:, 0]], axis=-1).reshape(HEADS * H, 2)
    ).astype(BF16))
    CT2 = (HEADS * H) // 128
    F2p = F2 + 2
    w2_tiled = tile128(np.ascontiguousarray(W2).astype(BF16))         # [128, CT2*F2]
    w2a = np.zeros((128, CT2 * F2p), BF16)
    for ct in range(CT2):
        w2a[:, ct * F2p: ct * F2p + F2] = w2_tiled[:, ct * F2: (ct + 1) * F2]
    w2t = np.ascontiguousarray(W2.T).astype(BF16)                     # [F2, HH]
    a2p = np.ascontiguousarray(
        np.stack([a2[:F2, 0], a2[F2:, 0]], axis=-1)
    ).astype(BF16)                                                    # [F2, 2]
    ident = np.eye(128, dtype=BF16)
    identf = np.eye(128, dtype=np.float32)

    adj = np.asarray(g) > 0
    in_maps = []
    for c in range(cfg.CORES):
        rows = slice(c * R, (c + 1) * R)
        mb = np.where(adj[rows].T, 0.0, MASKBIAS).astype(BF16)        # [N, R]
        in_maps.append({
            "xt": xt_t, "xtloc": tile128(np.ascontiguousarray(XT[:, rows])),
            "mb": tile128(np.ascontiguousarray(mb)),
            "w1c": w1c, "w1t": w1t, "a1p": a1p,
            "w2": w2a, "w2t": w2t, "a2p": a2p,
            "ident": ident, "identf": identf,
        })
    return in_maps


_NC_CACHE = {}


def get_compiled(cfg: Cfg):
    nc = _NC_CACHE.get(cfg)
    if nc is None:
        nc = build_gat_nc(cfg)
        _NC_CACHE[cfg] = nc
    return nc


def kernel(g, inputs, W1, a1, W2, a2):
    cfg = FULL
    nc = get_compiled(cfg)
    in_maps = host_prep(cfg, g, inputs, W1, a1, W2, a2)
    res = run_bass_kernel_spmd(nc, in_maps, core_ids=list(range(cfg.CORES)))
    out = np.concatenate(
        [np.asarray(res.results[c]["out"], np.float32) for c in range(cfg.CORES)],
        axis=0,
    )
    return out



# revision 41
# speedup vs baseline: 1.1272x; 1.1272x over previous
"""GAT (2-layer graph attention network) on 8 Trainium2 NeuronCores.

Strategy (1D node partition): each core owns R = N/8 rows (nodes).

Layer 1:
  - Wh plus the per-node score projections s- = Wh @ a_minus are computed
    from LOCAL rows only (one fused matmul chain per row tile; the
    projection weights W@a_half are precomputed on host), packed into an
    AllGather payload laid out [Wh_h0 |1| Wh_h1 |1| Wh_h2 |1| Wh_h3 |1|
    s-_0..3 | pad] so that each head's aggregation can stream a contiguous
    [Wh_h | ones] block.
  - Scores e[j, i] = leaky_relu(s+_i + s-_j + maskbias) are built by a
    fused custom DVE op per tile (or, for a tunable subset of tiles, by a
    native add + Activation-engine Lrelu-with-bias to balance engine load);
    exp on the scalar engine in GRP-tile batches.
  - Aggregation uses z as the matmul STATIONARY operand streaming the
    gathered [Wh_h | 1] columns: PSUM picks up both the softmax numerator
    (128 cols) and the denominator (col 128) in one pass — no separate
    denominator matmuls.  Normalize + elu then work per i-tile with
    [128,1] reciprocals and per-partition scalars.
Between layers: AllGather of [Wh2(64) | 1 | s2-] payload (68 cols/tile).
Layer 2: same fused-score pipeline; denominator rides as column 64 of the
stationary operand (m=65 <= 128).

Numerics: matmuls in bf16 (fp32 PSUM accumulate); mask handled as additive
-100 before leaky_relu: masked contribution < 1e-8 relative.
"""

import math
from contextlib import ExitStack
from dataclasses import dataclass

import numpy as np
import ml_dtypes

import concourse.bass as bass
import concourse.mybir as mybir
import concourse.tile as tile
from concourse import bacc
from concourse.bass_utils import run_bass_kernel_spmd

BF16 = ml_dtypes.bfloat16
ALPHA = 0.2
MASKBIAS = -100.0

# --------------------------------------------------------------------------
# Custom fused DVE ops (registered into concourse.dve_ops at import time)
# --------------------------------------------------------------------------

import concourse.dve_ops as dve_ops
from concourse.dve_spec import (
    Spec, Src0, Src1, C0, Zero, lower, maxx, select, _has_src1,
)
from concourse.dve_uop import DveOpSpec


def _make_specs():
    # out = max(y, alpha*y), y = (in0 + s0) + in1
    #   in0 = s1 broadcast [P, R]; s0 = s2 per-partition [P, 1];
    #   in1 = additive mask bias {0, -100}; imm2 = alpha
    from concourse.dve_spec import C2
    _y = (Src0 + C0) + Src1

    def _score_ref(in0, in1, s0, s1, imm2):
        y = in0.astype(np.float32) + s0 + in1.astype(np.float32)
        return np.maximum(y, y * imm2)

    score = Spec(body=maxx(_y, _y * C2), reference=_score_ref)

    # out = in0 > 0 ? in0 : in1 - s0   (elu with in1 = exp(in0), s0 = 1.0)
    def _elu_ref(in0, in1, s0, s1, imm2):
        x = in0.astype(np.float32)
        return np.where(x > 0, x, in1.astype(np.float32) - s0)

    elu = Spec(body=select(Src0 > Zero, Src0, Src1 - C0), reference=_elu_ref)
    return score, elu


def _register(name, spec):
    if name in dve_ops._SUB_OPCODE_FOR_NAME:
        for op in dve_ops.OPS:
            if op.name == name:
                return op
    row = max(dve_ops._SUB_OPCODE_FOR_NAME.values()) + 1
    assert row < 0x20
    shas = {}
    for ver in ("v3", "v4"):
        uops = lower(spec, ver=ver)
        shas[ver] = DveOpSpec(
            name=name, opcode=row, uops=uops, rd1_en=_has_src1(spec)
        ).sha(ver)
    op = dve_ops.DveOp(name, spec, subdim=False, uops_sha=shas)
    dve_ops.OPS.append(op)
    dve_ops.CUSTOM_DVE_SPECS[name] = spec
    dve_ops._SUB_OPCODE_FOR_NAME[name] = row
    return op


_SCORE_SPEC, _ELU_SPEC = _make_specs()
SCORE_LRELU = _register("SCORE_LRELU_GAT", _SCORE_SPEC)
ELU_SEL = _register("ELU_SEL_GAT", _ELU_SPEC)


# --------------------------------------------------------------------------
# Kernel configuration
# --------------------------------------------------------------------------

@dataclass(frozen=True)
class Cfg:
    N: int = 4096      # nodes
    C: int = 512       # input feature dim
    H: int = 128       # hidden per head (must be 128)
    HEADS: int = 4
    F2: int = 64       # output dim
    CORES: int = 8
    GRP: int = 8       # j-tiles per batched exp

    @property
    def R(self): return self.N // self.CORES          # rows per core
    @property
    def JT(self): return self.N // 128                # j tiles
    @property
    def CT(self): return self.C // 128                # input-feature tiles
    @property
    def HH(self): return self.HEADS * self.H          # layer-1 out features
    @property
    def CT2(self): return self.HH // 128              # layer-2 contraction tiles
    @property
    def RT(self): return self.R // 128                # local row tiles
    @property
    def PW1(self): return self.HEADS * 129 + self.HEADS  # 520 payload cols
    @property
    def PAY(self): return self.F2 + 4                 # L2 payload (64|1|s2|pad)
    @property
    def NG(self): return self.JT // self.GRP


FULL = Cfg()


# --------------------------------------------------------------------------
# Device program
# --------------------------------------------------------------------------

def build_gat_nc(cfg: Cfg, collective: bool = True, iters: int = 1,
                 loop_iters: int = 0, phases: str = "full",
                 debug_dump: bool = False):
    dt = mybir.dt.bfloat16
    f32 = mybir.dt.float32
    add = mybir.AluOpType.add
    mult = mybir.AluOpType.mult
    bypass = mybir.AluOpType.bypass
    Exp = mybir.ActivationFunctionType.Exp
    Lrelu = mybir.ActivationFunctionType.Lrelu

    N, C, HEADS, F2, R = cfg.N, cfg.C, cfg.HEADS, cfg.F2, cfg.R
    JT, CT, HH, CT2, RT = cfg.JT, cfg.CT, cfg.HH, cfg.CT2, cfg.RT
    PW1, PAY, GRP, NG = cfg.PW1, cfg.PAY, cfg.GRP, cfg.NG

    def is_b(t):
        return t % bmod[0] == bmod[1]

    def is_p(t):
        return t % pmod[0] == pmod[1]

    nc = bacc.Bacc(
        "TRN2", target_bir_lowering=False, debug=False, num_devices=cfg.CORES
    )

    # ---- DRAM I/O -------------------------------------------------------
    # wpack = [w1cs (CT*PW1) | wsp (CT*HEADS) | w2a (CT2*PAY) | ident (128)]
    WP0 = CT * PW1
    WP1 = WP0 + CT * HEADS
    WP2 = WP1 + CT2 * PAY
    WPN = WP2 + 128
    xtl_d = nc.dram_tensor("xtloc", [128, CT * R], dt, kind="ExternalInput").ap()
    mb_d = nc.dram_tensor("mb", [128, JT * R], dt, kind="ExternalInput").ap()
    wpack_d = nc.dram_tensor("wpack", [128, WPN], dt, kind="ExternalInput").ap()
    idf_d = nc.dram_tensor("identf", [128, 128], f32, kind="ExternalInput").ap()
    out_d = nc.dram_tensor("out", [R, F2], f32, kind="ExternalOutput").ap()
    if debug_dump:
        dbg_s1b = nc.dram_tensor(
            "dbg_s1b", [128, HEADS * R], dt, kind="ExternalOutput").ap()
        dbg_smin = nc.dram_tensor(
            "dbg_smin", [128, HEADS * JT], f32, kind="ExternalOutput").ap()
        dbg_hloc = nc.dram_tensor(
            "dbg_hloc", [128, CT2 * R], dt, kind="ExternalOutput").ap()
        dbg_wh = nc.dram_tensor(
            "dbg_wh", [128, JT * PW1], dt, kind="ExternalOutput").ap()
        dbg_pagg = nc.dram_tensor(
            "dbg_pagg", [128, HEADS * 2048], f32, kind="ExternalOutput").ap()

    with tile.TileContext(nc) as tc, ExitStack() as ctx:
        const = ctx.enter_context(tc.tile_pool(name="const", bufs=1))
        work = ctx.enter_context(tc.tile_pool(name="work", bufs=3))
        wz = ctx.enter_context(tc.tile_pool(name="wz", bufs=3))
        psA = ctx.enter_context(tc.tile_pool(name="psA", bufs=1, space="PSUM"))
        pss = ctx.enter_context(tc.tile_pool(name="pss", bufs=1, space="PSUM"))
        ps2 = ctx.enter_context(tc.tile_pool(name="ps2", bufs=1, space="PSUM"))
        dram = ctx.enter_context(tc.tile_pool(name="dram", bufs=1, space="DRAM"))

        whsend_t = dram.tile([128, RT * PW1], dt)
        gsend_t = dram.tile([128, RT * PAY], dt)
        if cfg.CORES > 4:
            whfull_t = nc.dram_tensor(
                "whfull_sh", [cfg.CORES * 128, RT * PW1], dt,
                addr_space="Shared").ap()
            gfull_t = nc.dram_tensor(
                "gfull_sh", [cfg.CORES * 128, RT * PAY], dt,
                addr_space="Shared").ap()
        else:
            whfull_t = dram.tile([cfg.CORES * 128, RT * PW1], dt)
            gfull_t = dram.tile([cfg.CORES * 128, RT * PAY], dt)

        import contextlib
        loop_cm = (tc.For_i(0, loop_iters, 1) if loop_iters
                   else contextlib.nullcontext())
        with loop_cm:
          for _it in range(iters):
            # ---- input loads, all on the cheap Pool DMA queue, ordered so
            # cross-iteration WAR hazards release early-to-late ------------
            wp_sb = const.tile([128, WPN], dt)
            nc.gpsimd.dma_start(out=wp_sb[:, 0:WP1], in_=wpack_d[:, 0:WP1])
            w1cs_sb = wp_sb[:, 0:WP0]
            wsp_sb = wp_sb[:, WP0:WP1]
            w2a_sb = wp_sb[:, WP1:WP2]
            ident_sb = wp_sb[:, WP2:WPN]
            xtl_sb = const.tile([128, CT * R], dt)
            nc.gpsimd.dma_start(out=xtl_sb, in_=xtl_d)
            mb_sb = const.tile([128, JT * R], dt)
            _hm = JT * R // 16
            for _q in range(4):
                nc.gpsimd.dma_start(
                    out=mb_sb[:, _q * _hm: (_q + 1) * _hm],
                    in_=mb_d[:, _q * _hm: (_q + 1) * _hm])
            nc.gpsimd.dma_start(out=wp_sb[:, WP1:WPN], in_=wpack_d[:, WP1:WPN])
            identf_sb = const.tile([128, 128], f32)
            nc.gpsimd.dma_start(out=identf_sb, in_=idf_d)

            if phases == "dma":
                for rt in range(RT):
                    nc.sync.dma_start(out=out_d[rt * 128:(rt + 1) * 128, :],
                                      in_=identf_sb[:, 0:F2])
                continue

            # ---- local Wh + s- payload, AllGather ------------------------
            # psum matmul outputs stay within one 2KB bank: the 520-wide
            # chain splits at col 512.
            pay_sb = const.tile([128, RT * PW1], dt)
            for rt in range(RT):
                pwh = psA.tile([128, PW1], f32, tag="agg")
                for ct in range(CT):
                    lhs = xtl_sb[:, ct * R + rt * 128: ct * R + (rt + 1) * 128]
                    nc.tensor.matmul(
                        out=pwh[:, 0:512], lhsT=lhs,
                        rhs=w1cs_sb[:, ct * PW1: ct * PW1 + 512],
                        start=(ct == 0), stop=(ct == CT - 1),
                    )
                    nc.tensor.matmul(
                        out=pwh[:, 512:PW1], lhsT=lhs,
                        rhs=w1cs_sb[:, ct * PW1 + 512: (ct + 1) * PW1],
                        start=(ct == 0), stop=(ct == CT - 1),
                    )
                pay = pay_sb[:, rt * PW1: (rt + 1) * PW1]
                nc.scalar.copy(out=pay, in_=pwh)
                for h in range(HEADS):
                    nc.vector.memset(pay[:, h * 129 + 128: h * 129 + 129], 1.0)
            nc.sync.dma_start(out=whsend_t, in_=pay_sb)

            # s+ for local nodes, all heads in one m=4 chain; flatten the 4
            # head rows onto partition 0 with small pool-queue DMAs, then
            # broadcast each via PE ones-matmul + Act copy.
            psp = pss.tile([HEADS, R], f32, tag="tr")
            for ct in range(CT):
                nc.tensor.matmul(
                    out=psp,
                    lhsT=wsp_sb[:, ct * HEADS: (ct + 1) * HEADS],
                    rhs=xtl_sb[:, ct * R: (ct + 1) * R],
                    start=(ct == 0), stop=(ct == CT - 1),
                )
            s1r_sb = const.tile([HEADS, R], dt)
            nc.scalar.copy(out=s1r_sb, in_=psp)
            s1rf_sb = const.tile([1, HEADS * R], dt)
            for h in range(HEADS):
                nc.gpsimd.dma_start(
                    out=s1rf_sb[0:1, h * R: (h + 1) * R],
                    in_=s1r_sb[h: h + 1, :])
            ones_row = const.tile([1, 128], dt)
            nc.vector.memset(ones_row, 1.0)
            s1b_sb = const.tile([128, HEADS * R], dt)
            for h in range(HEADS):
                pbc = pss.tile([128, R], f32, tag="tr")
                nc.tensor.matmul(out=pbc, lhsT=ones_row,
                                 rhs=s1rf_sb[0:1, h * R: (h + 1) * R],
                                 start=True, stop=True)
                nc.scalar.copy(out=s1b_sb[:, h * R: (h + 1) * R], in_=pbc)

            if collective:
                nc.gpsimd.collective_compute(
                    "AllGather", bypass,
                    replica_groups=[list(range(cfg.CORES))],
                    ins=[whsend_t.opt()], outs=[whfull_t.opt()],
                )
            else:
                # timing proxy: 4 same-volume writes carry the send cost and
                # the dependency; chunk readbacks below read whfull (rows
                # beyond 512 are stale, values unused for timing).
                for cc in range(4):
                    nc.gpsimd.dma_start(
                        out=whfull_t[cc * 256: cc * 256 + 128, :],
                        in_=whsend_t[:, :])
            # readback in 4 chunked DMAs (2 cores per DMA) so L1 can start
            # on early tiles; smin (f32 s- scalars) extracted per chunk.
            wh_sb = const.tile([128, JT * PW1], dt)
            smin_sb = const.tile([128, HEADS * JT], f32)
            whf_v = whfull_t.rearrange("(c p) x -> p c x", p=128)
            _whq = [nc.sync, nc.sync, nc.sync, nc.sync]
            for cc in range(4):
                _whq[cc].dma_start(
                    out=wh_sb[:, cc * 2 * RT * PW1: (cc + 1) * 2 * RT * PW1]
                    .rearrange("p (c x) -> p c x", c=2),
                    in_=whf_v[:, cc * 2: cc * 2 + 2, :],
                )
                for h in range(HEADS):
                    nc.vector.tensor_copy(
                        out=smin_sb[:, h * JT + cc * 8: h * JT + (cc + 1) * 8]
                        .rearrange("p (t o) -> p t o", o=1),
                        in_=wh_sb.rearrange("p (t q) -> p t q", q=PW1)[
                            :, cc * 8: (cc + 1) * 8,
                            HEADS * 129 + h: HEADS * 129 + h + 1],
                    )

            for _q in range(4, 16):
                nc.gpsimd.dma_start(
                    out=mb_sb[:, _q * _hm: (_q + 1) * _hm],
                    in_=mb_d[:, _q * _hm: (_q + 1) * _hm])

            if phases == "wh":
                for rt in range(RT):
                    nc.sync.dma_start(out=out_d[rt * 128:(rt + 1) * 128, :],
                                      in_=identf_sb[:, 0:F2])
                continue

            # ---- layer 1: per-head attention + aggregation ----------------
            hloc_sb = const.tile([128, CT2 * R], dt)  # h_local^T, feature-major
            # aggregation psum: one chain per 2KB bank — interleaved
            # accumulation chains sharing a bank corrupt each other.
            aoff = [0, 512, 1024, 1536]
            for h in range(HEADS):
                s1b = s1b_sb[:, h * R: (h + 1) * R]
                pagg = psA.tile([128, 2048], f32, tag="agg")
                for g in range(NG):
                    ug = wz.tile([128, GRP * R], dt, tag="ug")
                    zg = wz.tile([128, GRP * R], dt, tag="zg")
                    for k in range(GRP):
                        t = g * GRP + k
                        nc.vector._custom_dve(
                            SCORE_LRELU,
                            out=ug[:, k * R: (k + 1) * R],
                            in0=s1b,
                            in1=mb_sb[:, t * R: (t + 1) * R],
                            s0=smin_sb[:, h * JT + t: h * JT + t + 1],
                            s1=0.0, imm2=ALPHA,
                        )
                    nc.scalar.activation(out=zg[:, :], in_=ug[:, :], func=Exp)
                    for k in range(GRP):
                        t = g * GRP + k
                        for i4 in range(4):
                            nc.tensor.matmul(
                                out=pagg[:, aoff[i4]: aoff[i4] + 129],
                                lhsT=zg[:, k * R + i4 * 128: k * R + (i4 + 1) * 128],
                                rhs=wh_sb[:, t * PW1 + h * 129:
                                          t * PW1 + (h + 1) * 129],
                                start=(t == 0), stop=(t == JT - 1),
                            )
                if debug_dump:
                    paggf = const.tile([128, 2048], f32, name=f"dbgp{h}")
                    nc.vector.tensor_copy(out=paggf, in_=pagg)
                    nc.sync.dma_start(
                        out=dbg_pagg[:, h * 2048: (h + 1) * 2048], in_=paggf)
                # normalize + elu + transpose -> hloc tile for this head
                for i4 in range(4):
                    rcp = work.tile([128, 1], f32, tag="rcp")
                    nc.vector.reciprocal(
                        out=rcp, in_=pagg[:, aoff[i4] + 128: aoff[i4] + 129])
                    hu = work.tile([128, 128], dt, tag="hu")
                    nc.vector.tensor_scalar(
                        out=hu, in0=pagg[:, aoff[i4]: aoff[i4] + 128],
                        scalar1=rcp, scalar2=0.0, op0=mult, op1=bypass,
                    )
                    eh = work.tile([128, 128], dt, tag="eh")
                    nc.scalar.activation(out=eh, in_=hu, func=Exp)
                    helu = work.tile([128, 128], dt, tag="helu")
                    nc.vector._custom_dve(
                        ELU_SEL, out=helu, in0=hu, in1=eh,
                        s0=1.0, s1=0.0, imm2=0.0,
                    )
                    pT = pss.tile([128, 128], dt, tag="tr")
                    nc.tensor.transpose(out=pT, in_=helu, identity=ident_sb)
                    nc.scalar.copy(
                        out=hloc_sb[:, h * R + i4 * 128: h * R + (i4 + 1) * 128],
                        in_=pT)

            if debug_dump:
                nc.sync.dma_start(out=dbg_s1b, in_=s1b_sb)
                nc.sync.dma_start(out=dbg_smin, in_=smin_sb)
                nc.sync.dma_start(out=dbg_hloc, in_=hloc_sb)
                nc.sync.dma_start(out=dbg_wh, in_=wh_sb)

            if phases == "l1":
                for rt in range(RT):
                    nc.sync.dma_start(out=out_d[rt * 128:(rt + 1) * 128, :],
                                      in_=identf_sb[:, 0:F2])
                continue

            # ---- layer-2 local projections + gather payload ---------------
            # w2a cols per ct2: [W2(64) | w2p | w2m | pad]; pW = [Wh2 | s+ | s-]
            sp2_sb = work.tile([128, RT], dt, tag="sp2")
            gs_sb = const.tile([128, RT * PAY], dt)
            for rt in range(RT):
                pW = pss.tile([128, PAY], f32, tag="pw", bufs=2)
                for ct in range(CT2):
                    nc.tensor.matmul(
                        out=pW,
                        lhsT=hloc_sb[:, ct * R + rt * 128: ct * R + (rt + 1) * 128],
                        rhs=w2a_sb[:, ct * PAY: (ct + 1) * PAY],
                        start=(ct == 0), stop=(ct == CT2 - 1),
                    )
                gs = gs_sb[:, rt * PAY: (rt + 1) * PAY]
                nc.scalar.copy(out=gs[:, 0:F2], in_=pW[:, 0:F2])
                nc.vector.memset(gs[:, F2: F2 + 1], 1.0)
                nc.vector.tensor_copy(
                    out=gs[:, F2 + 1: F2 + 2], in_=pW[:, F2 + 1: F2 + 2])
                nc.vector.tensor_copy(
                    out=sp2_sb[:, rt: rt + 1], in_=pW[:, F2: F2 + 1])
            nc.sync.dma_start(out=gsend_t, in_=gs_sb)

            # local s+ -> s1b2 broadcast (no gather dependency): transpose
            # each sp2 column to a partition-0 row, then one ones-matmul.
            s1r2f_sb = const.tile([1, R], dt)
            for rt in range(RT):
                pt1 = pss.tile([1, 128], dt, tag="tr")
                nc.tensor.transpose(
                    out=pt1, in_=sp2_sb[:, rt: rt + 1], identity=ident_sb)
                nc.vector.tensor_copy(
                    out=s1r2f_sb[0:1, rt * 128: (rt + 1) * 128], in_=pt1)
            pbc2 = pss.tile([128, R], f32, tag="tr")
            nc.tensor.matmul(out=pbc2, lhsT=ones_row, rhs=s1r2f_sb,
                             start=True, stop=True)
            s1b2_sb = const.tile([128, R], dt)
            nc.scalar.copy(out=s1b2_sb, in_=pbc2)

            if collective:
                nc.gpsimd.collective_compute(
                    "AllGather", bypass,
                    replica_groups=[list(range(cfg.CORES))],
                    ins=[gsend_t.opt()], outs=[gfull_t.opt()],
                )
            else:
                nc.gpsimd.dma_start(
                    out=gfull_t[0:128, :], in_=gsend_t[:, :])

            gf_sb = const.tile([128, JT * PAY], dt)
            gff_v = gfull_t.rearrange("(c p) x -> p c x", p=128)
            _gfq = [nc.sync, nc.sync, nc.sync, nc.sync]
            for cc in range(4):
                _gfq[cc].dma_start(
                    out=gf_sb[:, cc * 2 * RT * PAY: (cc + 1) * 2 * RT * PAY]
                    .rearrange("p (c x) -> p c x", c=2),
                    in_=gff_v[:, cc * 2: cc * 2 + 2, :],
                )
            s2pf = const.tile([128, JT], f32)
            nc.vector.tensor_copy(
                out=s2pf[:, :].rearrange("p (t o) -> p t o", o=1),
                in_=gf_sb.rearrange("p (t q) -> p t q", q=PAY)[
                    :, :, F2 + 1: F2 + 2],
            )

            # ---- layer-2 attention + aggregation --------------------------
            psum2 = ps2.tile([F2 + 1, R], f32)
            for g in range(NG):
                ug = wz.tile([128, GRP * R], dt, tag="ug")
                zg = wz.tile([128, GRP * R], dt, tag="zg")
                for k in range(GRP):
                    t = g * GRP + k
                    nc.vector._custom_dve(
                        SCORE_LRELU,
                        out=ug[:, k * R: (k + 1) * R],
                        in0=s1b2_sb,
                        in1=mb_sb[:, t * R: (t + 1) * R],
                        s0=s2pf[:, t: t + 1], s1=0.0, imm2=ALPHA,
                    )
                nc.scalar.activation(out=zg[:, :], in_=ug[:, :], func=Exp)
                for k in range(GRP):
                    t = g * GRP + k
                    nc.tensor.matmul(
                        out=psum2,
                        lhsT=gf_sb[:, t * PAY: t * PAY + F2 + 1],
                        rhs=zg[:, k * R: (k + 1) * R],
                        start=(t == 0), stop=(t == JT - 1),
                    )

            # ---- finalize: transpose, normalize, store --------------------
            o2 = const.tile([F2 + 1, R], f32)
            nc.scalar.copy(out=o2, in_=psum2)
            of_sb = const.tile([128, RT * F2], f32)
            for rt in range(RT):
                pT2 = pss.tile([128, F2 + 1], f32, tag="pw", bufs=2)
                nc.tensor.transpose(
                    out=pT2,
                    in_=o2[:, rt * 128: (rt + 1) * 128],
                    identity=identf_sb[0: F2 + 1, 0: F2 + 1],
                )
                rc = work.tile([128, 1], f32, tag="rc")
                nc.vector.reciprocal(out=rc, in_=pT2[:, F2: F2 + 1])
                nc.vector.tensor_scalar(
                    out=of_sb[:, rt * F2: (rt + 1) * F2],
                    in0=pT2[:, 0:F2], scalar1=rc, scalar2=0.0,
                    op0=mult, op1=bypass,
                )
            nc.sync.dma_start(
                out=out_d.rearrange("(rt p) f -> p rt f", p=128),
                in_=of_sb.rearrange("p (rt f) -> p rt f", rt=RT),
            )

    nc.compile()
    return nc


# --------------------------------------------------------------------------
# Host-side prep / sharding
# --------------------------------------------------------------------------

def host_prep(cfg: Cfg, g, inputs, W1, a1, W2, a2):
    N, C, H, HEADS, F2, R = cfg.N, cfg.C, cfg.H, cfg.HEADS, cfg.F2, cfg.R
    CT, CT2, PW1, PAY = cfg.CT, cfg.CT2, cfg.PW1, cfg.PAY
    X = np.asarray(inputs, np.float32)
    W1 = np.asarray(W1, np.float32)
    a1 = np.asarray(a1, np.float32)
    W2 = np.asarray(W2, np.float32)
    a2 = np.asarray(a2, np.float32)

    def tile128(A):
        # [k*128, cols] row-major -> partition-major [128, k*cols]
        k = A.shape[0] // 128
        return np.ascontiguousarray(
            A.reshape(k, 128, A.shape[1]).transpose(1, 0, 2).reshape(128, -1)
        )

    XT = np.ascontiguousarray(X.T)                                    # [C, N]

    # w1cs: per ct block [128, PW1]: [W1_h0 |0| W1_h1 |0| W1_h2 |0| W1_h3 |0|
    #                                 ws1m_0..3]
    ws1p = np.stack([W1[h] @ a1[h, :H, 0] for h in range(HEADS)], axis=1)  # [C,4]
    ws1m = np.stack([W1[h] @ a1[h, H:, 0] for h in range(HEADS)], axis=1)  # [C,4]
    w1cs = np.zeros((C, PW1), np.float32)
    for h in range(HEADS):
        w1cs[:, h * 129: h * 129 + 128] = W1[h]
    w1cs[:, HEADS * 129:] = ws1m
    w1cs_t = tile128(w1cs.astype(BF16))
    wsp_t = tile128(ws1p.astype(BF16))

    # w2a: per ct2 block [128, PAY]: [W2(64) | w2p | w2m | pad]
    w2p = W2 @ a2[:F2, 0]                                             # [HH]
    w2m = W2 @ a2[F2:, 0]                                             # [HH]
    w2a = np.zeros((HEADS * H, PAY), np.float32)
    w2a[:, :F2] = W2
    w2a[:, F2] = w2p
    w2a[:, F2 + 1] = w2m
    w2a_t = tile128(w2a.astype(BF16))

    ident = np.eye(128, dtype=BF16)
    identf = np.eye(128, dtype=np.float32)
    wpack = np.concatenate([w1cs_t, wsp_t, w2a_t, ident], axis=1)

    adj = np.asarray(g) > 0
    in_maps = []
    for c in range(cfg.CORES):
        rows = slice(c * R, (c + 1) * R)
        mb = np.where(adj[rows].T, 0.0, MASKBIAS).astype(BF16)        # [N, R]
        in_maps.append({
            "xtloc": tile128(np.ascontiguousarray(XT[:, rows]).astype(BF16)),
            "mb": tile128(np.ascontiguousarray(mb)),
            "wpack": wpack, "identf": identf,
        })
    return in_maps


_NC_CACHE = {}


def get_compiled(cfg: Cfg):
    nc = _NC_CACHE.get(cfg)
    if nc is None:
        nc = build_gat_nc(cfg)
        _NC_CACHE[cfg] = nc
    return nc


def kernel(g, inputs, W1, a1, W2, a2):
    cfg = FULL
    nc = get_compiled(cfg)
    in_maps = host_prep(cfg, g, inputs, W1, a1, W2, a2)
    res = run_bass_kernel_spmd(nc, in_maps, core_ids=list(range(cfg.CORES)))
    out = np.concatenate(
        [np.asarray(res.results[c]["out"], np.float32) for c in range(cfg.CORES)],
        axis=0,
    )
    return out


# revision 43
# speedup vs baseline: 1.2141x; 1.0771x over previous
"""GAT (2-layer graph attention network) on 8 Trainium2 NeuronCores.

Strategy (1D node partition): each core owns R = N/8 rows (nodes).

Layer 1:
  - Wh plus the per-node score projections s- = Wh @ a_minus are computed
    from LOCAL rows only (one fused matmul chain per row tile; the
    projection weights W@a_half are precomputed on host), packed into an
    AllGather payload laid out [Wh_h0 |1| Wh_h1 |1| Wh_h2 |1| Wh_h3 |1|
    s-_0..3 | pad] so that each head's aggregation can stream a contiguous
    [Wh_h | ones] block.
  - Scores e[j, i] = leaky_relu(s+_i + s-_j + maskbias) are built by a
    fused custom DVE op per tile (or, for a tunable subset of tiles, by a
    native add + Activation-engine Lrelu-with-bias to balance engine load);
    exp on the scalar engine in GRP-tile batches.
  - Aggregation uses z as the matmul STATIONARY operand streaming the
    gathered [Wh_h | 1] columns: PSUM picks up both the softmax numerator
    (128 cols) and the denominator (col 128) in one pass — no separate
    denominator matmuls.  Normalize + elu then work per i-tile with
    [128,1] reciprocals and per-partition scalars.
Between layers: AllGather of [Wh2(64) | 1 | s2-] payload (68 cols/tile).
Layer 2: same fused-score pipeline; denominator rides as column 64 of the
stationary operand (m=65 <= 128).

Numerics: matmuls in bf16 (fp32 PSUM accumulate); mask handled as additive
-100 before leaky_relu: masked contribution < 1e-8 relative.
"""

import math
from contextlib import ExitStack
from dataclasses import dataclass

import numpy as np
import ml_dtypes

import concourse.bass as bass
import concourse.mybir as mybir
import concourse.tile as tile
from concourse import bacc
from concourse.bass_utils import run_bass_kernel_spmd

BF16 = ml_dtypes.bfloat16
ALPHA = 0.2
MASKBIAS = -100.0

# --------------------------------------------------------------------------
# Custom fused DVE ops (registered into concourse.dve_ops at import time)
# --------------------------------------------------------------------------

import concourse.dve_ops as dve_ops
from concourse.dve_spec import (
    Spec, Src0, Src1, C0, Zero, lower, maxx, select, _has_src1,
)
from concourse.dve_uop import DveOpSpec


def _make_specs():
    # out = max(y, alpha*y), y = (in0 + s0) + in1
    #   in0 = s1 broadcast [P, R]; s0 = s2 per-partition [P, 1];
    #   in1 = additive mask bias {0, -100}; imm2 = alpha
    from concourse.dve_spec import C2
    _y = (Src0 + C0) + Src1

    def _score_ref(in0, in1, s0, s1, imm2):
        y = in0.astype(np.float32) + s0 + in1.astype(np.float32)
        return np.maximum(y, y * imm2)

    score = Spec(body=maxx(_y, _y * C2), reference=_score_ref)

    # out = in0 > 0 ? in0 : in1 - s0   (elu with in1 = exp(in0), s0 = 1.0)
    def _elu_ref(in0, in1, s0, s1, imm2):
        x = in0.astype(np.float32)
        return np.where(x > 0, x, in1.astype(np.float32) - s0)

    elu = Spec(body=select(Src0 > Zero, Src0, Src1 - C0), reference=_elu_ref)
    return score, elu


def _register(name, spec):
    if name in dve_ops._SUB_OPCODE_FOR_NAME:
        for op in dve_ops.OPS:
            if op.name == name:
                return op
    row = max(dve_ops._SUB_OPCODE_FOR_NAME.values()) + 1
    assert row < 0x20
    shas = {}
    for ver in ("v3", "v4"):
        uops = lower(spec, ver=ver)
        shas[ver] = DveOpSpec(
            name=name, opcode=row, uops=uops, rd1_en=_has_src1(spec)
        ).sha(ver)
    op = dve_ops.DveOp(name, spec, subdim=False, uops_sha=shas)
    dve_ops.OPS.append(op)
    dve_ops.CUSTOM_DVE_SPECS[name] = spec
    dve_ops._SUB_OPCODE_FOR_NAME[name] = row
    return op


_SCORE_SPEC, _ELU_SPEC = _make_specs()
SCORE_LRELU = _register("SCORE_LRELU_GAT", _SCORE_SPEC)
ELU_SEL = _register("ELU_SEL_GAT", _ELU_SPEC)


# --------------------------------------------------------------------------
# Kernel configuration
# --------------------------------------------------------------------------

@dataclass(frozen=True)
class Cfg:
    N: int = 4096      # nodes
    C: int = 512       # input feature dim
    H: int = 128       # hidden per head (must be 128)
    HEADS: int = 4
    F2: int = 64       # output dim
    CORES: int = 8
    GRP: int = 8       # j-tiles per batched exp

    @property
    def R(self): return self.N // self.CORES          # rows per core
    @property
    def JT(self): return self.N // 128                # j tiles
    @property
    def CT(self): return self.C // 128                # input-feature tiles
    @property
    def HH(self): return self.HEADS * self.H          # layer-1 out features
    @property
    def CT2(self): return self.HH // 128              # layer-2 contraction tiles
    @property
    def RT(self): return self.R // 128                # local row tiles
    @property
    def PW1(self): return self.HEADS * 129 + self.HEADS  # 520 payload cols
    @property
    def PAY(self): return self.F2 + 4                 # L2 payload (64|1|s2|pad)
    @property
    def NG(self): return self.JT // self.GRP


FULL = Cfg()


# --------------------------------------------------------------------------
# Device program
# --------------------------------------------------------------------------

def build_gat_nc(cfg: Cfg, collective: bool = True, iters: int = 1,
                 loop_iters: int = 0, phases: str = "full",
                 debug_dump: bool = False):
    dt = mybir.dt.bfloat16
    f32 = mybir.dt.float32
    add = mybir.AluOpType.add
    mult = mybir.AluOpType.mult
    bypass = mybir.AluOpType.bypass
    Exp = mybir.ActivationFunctionType.Exp
    Lrelu = mybir.ActivationFunctionType.Lrelu

    N, C, HEADS, F2, R = cfg.N, cfg.C, cfg.HEADS, cfg.F2, cfg.R
    JT, CT, HH, CT2, RT = cfg.JT, cfg.CT, cfg.HH, cfg.CT2, cfg.RT
    PW1, PAY, GRP, NG = cfg.PW1, cfg.PAY, cfg.GRP, cfg.NG

    nc = bacc.Bacc(
        "TRN2", target_bir_lowering=False, debug=False, num_devices=cfg.CORES
    )

    # ---- DRAM I/O -------------------------------------------------------
    # wpack = [w1cs (CT*PW1) | wsp (CT*HEADS) | w2a (CT2*PAY) | ident (128)]
    WP0 = CT * PW1
    WP1 = WP0 + CT * HEADS
    WP2 = WP1 + CT2 * PAY
    WPN = WP2 + 128
    xtl_d = nc.dram_tensor("xtloc", [128, CT * R], dt, kind="ExternalInput").ap()
    mb_d = nc.dram_tensor("mb", [128, JT * R], dt, kind="ExternalInput").ap()
    wpack_d = nc.dram_tensor("wpack", [128, WPN], dt, kind="ExternalInput").ap()
    idf_d = nc.dram_tensor("identf", [128, 128], f32, kind="ExternalInput").ap()
    out_d = nc.dram_tensor("out", [R, F2], f32, kind="ExternalOutput").ap()
    if debug_dump:
        dbg_s1b = nc.dram_tensor(
            "dbg_s1b", [128, HEADS * R], dt, kind="ExternalOutput").ap()
        dbg_smin = nc.dram_tensor(
            "dbg_smin", [128, HEADS * JT], f32, kind="ExternalOutput").ap()
        dbg_hloc = nc.dram_tensor(
            "dbg_hloc", [128, CT2 * R], dt, kind="ExternalOutput").ap()
        dbg_wh = nc.dram_tensor(
            "dbg_wh", [128, JT * PW1], dt, kind="ExternalOutput").ap()
        dbg_pagg = nc.dram_tensor(
            "dbg_pagg", [128, HEADS * 2048], f32, kind="ExternalOutput").ap()

    with tile.TileContext(nc) as tc, ExitStack() as ctx:
        const = ctx.enter_context(tc.tile_pool(name="const", bufs=1))
        work = ctx.enter_context(tc.tile_pool(name="work", bufs=3))
        wz = ctx.enter_context(tc.tile_pool(name="wz", bufs=3))
        psA = ctx.enter_context(tc.tile_pool(name="psA", bufs=1, space="PSUM"))
        pss = ctx.enter_context(tc.tile_pool(name="pss", bufs=1, space="PSUM"))
        ps2 = ctx.enter_context(tc.tile_pool(name="ps2", bufs=1, space="PSUM"))
        dram = ctx.enter_context(tc.tile_pool(name="dram", bufs=1, space="DRAM"))

        whsend_t = dram.tile([128, RT * PW1], dt)
        gsend_t = dram.tile([128, RT * PAY], dt)
        if cfg.CORES > 4:
            whfull_t = nc.dram_tensor(
                "whfull_sh", [cfg.CORES * 128, RT * PW1], dt,
                addr_space="Shared").ap()
            gfull_t = nc.dram_tensor(
                "gfull_sh", [cfg.CORES * 128, RT * PAY], dt,
                addr_space="Shared").ap()
        else:
            whfull_t = dram.tile([cfg.CORES * 128, RT * PW1], dt)
            gfull_t = dram.tile([cfg.CORES * 128, RT * PAY], dt)

        import contextlib
        loop_cm = (tc.For_i(0, loop_iters, 1) if loop_iters
                   else contextlib.nullcontext())
        with loop_cm:
          for _it in range(iters):
            # ---- input loads, all on the cheap Pool DMA queue, ordered so
            # cross-iteration WAR hazards release early-to-late ------------
            wp_sb = const.tile([128, WPN], dt)
            nc.gpsimd.dma_start(out=wp_sb[:, 0:WP1], in_=wpack_d[:, 0:WP1])
            w1cs_sb = wp_sb[:, 0:WP0]
            wsp_sb = wp_sb[:, WP0:WP1]
            w2a_sb = wp_sb[:, WP1:WP2]
            ident_sb = wp_sb[:, WP2:WPN]
            xtl_sb = const.tile([128, CT * R], dt)
            nc.gpsimd.dma_start(out=xtl_sb, in_=xtl_d)
            mb_sb = const.tile([128, JT * R], dt)
            _hm = JT * R // 16
            for _q in range(4):
                nc.gpsimd.dma_start(
                    out=mb_sb[:, _q * _hm: (_q + 1) * _hm],
                    in_=mb_d[:, _q * _hm: (_q + 1) * _hm])
            nc.gpsimd.dma_start(out=wp_sb[:, WP1:WPN], in_=wpack_d[:, WP1:WPN])
            identf_sb = const.tile([128, 128], f32)
            nc.gpsimd.dma_start(out=identf_sb, in_=idf_d)

            if phases == "dma":
                for rt in range(RT):
                    nc.sync.dma_start(out=out_d[rt * 128:(rt + 1) * 128, :],
                                      in_=identf_sb[:, 0:F2])
                continue

            # ---- local Wh + s- payload, AllGather ------------------------
            # psum matmul outputs stay within one 2KB bank: the 520-wide
            # chain splits at col 512.
            pay_sb = const.tile([128, RT * PW1], dt)
            for rt in range(RT):
                pwh = psA.tile([128, PW1], f32, tag="agg")
                for ct in range(CT):
                    lhs = xtl_sb[:, ct * R + rt * 128: ct * R + (rt + 1) * 128]
                    nc.tensor.matmul(
                        out=pwh[:, 0:512], lhsT=lhs,
                        rhs=w1cs_sb[:, ct * PW1: ct * PW1 + 512],
                        start=(ct == 0), stop=(ct == CT - 1),
                    )
                    nc.tensor.matmul(
                        out=pwh[:, 512:PW1], lhsT=lhs,
                        rhs=w1cs_sb[:, ct * PW1 + 512: (ct + 1) * PW1],
                        start=(ct == 0), stop=(ct == CT - 1),
                    )
                pay = pay_sb[:, rt * PW1: (rt + 1) * PW1]
                nc.scalar.copy(out=pay, in_=pwh)
                for h in range(HEADS):
                    nc.vector.memset(pay[:, h * 129 + 128: h * 129 + 129], 1.0)
            nc.sync.dma_start(out=whsend_t, in_=pay_sb)

            # s+ for local nodes, all heads in one m=4 chain; flatten the 4
            # head rows onto partition 0 with small pool-queue DMAs, then
            # broadcast each via PE ones-matmul + Act copy.
            psp = pss.tile([HEADS, R], f32, tag="tr")
            for ct in range(CT):
                nc.tensor.matmul(
                    out=psp,
                    lhsT=wsp_sb[:, ct * HEADS: (ct + 1) * HEADS],
                    rhs=xtl_sb[:, ct * R: (ct + 1) * R],
                    start=(ct == 0), stop=(ct == CT - 1),
                )
            s1r_sb = const.tile([HEADS, R], dt)
            nc.scalar.copy(out=s1r_sb, in_=psp)
            s1rf_sb = const.tile([1, HEADS * R], dt)
            for h in range(HEADS):
                nc.scalar.dma_start(
                    out=s1rf_sb[0:1, h * R: (h + 1) * R],
                    in_=s1r_sb[h: h + 1, :])
            ones_row = const.tile([1, 128], dt)
            nc.vector.memset(ones_row, 1.0)
            s1b_sb = const.tile([128, HEADS * R], dt)
            for h in range(HEADS):
                pbc = pss.tile([128, R], f32, tag="tr")
                nc.tensor.matmul(out=pbc, lhsT=ones_row,
                                 rhs=s1rf_sb[0:1, h * R: (h + 1) * R],
                                 start=True, stop=True)
                nc.scalar.copy(out=s1b_sb[:, h * R: (h + 1) * R], in_=pbc)

            if collective:
                nc.gpsimd.collective_compute(
                    "AllGather", bypass,
                    replica_groups=[list(range(cfg.CORES))],
                    ins=[whsend_t.opt()], outs=[whfull_t.opt()],
                )
            else:
                # timing proxy: 4 same-volume writes carry the send cost and
                # the dependency; chunk readbacks below read whfull (rows
                # beyond 512 are stale, values unused for timing).
                for cc in range(4):
                    nc.gpsimd.dma_start(
                        out=whfull_t[cc * 256: cc * 256 + 128, :],
                        in_=whsend_t[:, :])
            # readback in 4 chunked DMAs (2 cores per DMA) so L1 can start
            # on early tiles; smin (f32 s- scalars) extracted per chunk.
            wh_sb = const.tile([128, JT * PW1], dt)
            smin_sb = const.tile([128, HEADS * JT], f32)
            whf_v = whfull_t.rearrange("(c p) x -> p c x", p=128)
            _whq = [nc.sync, nc.sync, nc.sync, nc.sync]
            for cc in range(4):
                _whq[cc].dma_start(
                    out=wh_sb[:, cc * 2 * RT * PW1: (cc + 1) * 2 * RT * PW1]
                    .rearrange("p (c x) -> p c x", c=2),
                    in_=whf_v[:, cc * 2: cc * 2 + 2, :],
                )
                for h in range(HEADS):
                    nc.vector.tensor_copy(
                        out=smin_sb[:, h * JT + cc * 8: h * JT + (cc + 1) * 8]
                        .rearrange("p (t o) -> p t o", o=1),
                        in_=wh_sb.rearrange("p (t q) -> p t q", q=PW1)[
                            :, cc * 8: (cc + 1) * 8,
                            HEADS * 129 + h: HEADS * 129 + h + 1],
                    )

            for _q in range(4, 16):
                nc.gpsimd.dma_start(
                    out=mb_sb[:, _q * _hm: (_q + 1) * _hm],
                    in_=mb_d[:, _q * _hm: (_q + 1) * _hm])

            if phases == "wh":
                for rt in range(RT):
                    nc.sync.dma_start(out=out_d[rt * 128:(rt + 1) * 128, :],
                                      in_=identf_sb[:, 0:F2])
                continue

            # ---- layer 1: per-head attention + aggregation ----------------
            hloc_sb = const.tile([128, CT2 * R], dt)  # h_local^T, feature-major
            # aggregation psum: one chain per 2KB bank — interleaved
            # accumulation chains sharing a bank corrupt each other.
            aoff = [0, 512, 1024, 1536]
            for h in range(HEADS):
                s1b = s1b_sb[:, h * R: (h + 1) * R]
                pagg = psA.tile([128, 2048], f32, tag="agg")
                for g in range(NG):
                    ug = wz.tile([128, GRP * R], dt, tag="ug")
                    zg = wz.tile([128, GRP * R], dt, tag="zg")
                    for k in range(GRP):
                        t = g * GRP + k
                        nc.vector._custom_dve(
                            SCORE_LRELU,
                            out=ug[:, k * R: (k + 1) * R],
                            in0=s1b,
                            in1=mb_sb[:, t * R: (t + 1) * R],
                            s0=smin_sb[:, h * JT + t: h * JT + t + 1],
                            s1=0.0, imm2=ALPHA,
                        )
                    half = GRP * R // 2
                    nc.scalar.activation(out=zg[:, 0:half], in_=ug[:, 0:half],
                                         func=Exp)
                    nc.scalar.activation(out=zg[:, half:], in_=ug[:, half:],
                                         func=Exp)
                    for k in range(GRP):
                        t = g * GRP + k
                        for i4 in range(4):
                            nc.tensor.matmul(
                                out=pagg[:, aoff[i4]: aoff[i4] + 129],
                                lhsT=zg[:, k * R + i4 * 128: k * R + (i4 + 1) * 128],
                                rhs=wh_sb[:, t * PW1 + h * 129:
                                          t * PW1 + (h + 1) * 129],
                                start=(t == 0), stop=(t == JT - 1),
                            )
                if debug_dump:
                    paggf = const.tile([128, 2048], f32, name=f"dbgp{h}")
                    nc.vector.tensor_copy(out=paggf, in_=pagg)
                    nc.sync.dma_start(
                        out=dbg_pagg[:, h * 2048: (h + 1) * 2048], in_=paggf)
                # normalize + elu + transpose -> hloc tile for this head
                for i4 in range(4):
                    rcp = work.tile([128, 1], f32, tag="rcp")
                    nc.vector.reciprocal(
                        out=rcp, in_=pagg[:, aoff[i4] + 128: aoff[i4] + 129])
                    hu = work.tile([128, 128], f32, tag="hu")
                    nc.scalar.activation(
                        out=hu, in_=pagg[:, aoff[i4]: aoff[i4] + 128],
                        func=mybir.ActivationFunctionType.Copy, scale=rcp,
                    )
                    eh = work.tile([128, 128], dt, tag="eh")
                    nc.scalar.activation(out=eh, in_=hu, func=Exp)
                    helu = work.tile([128, 128], dt, tag="helu")
                    nc.vector._custom_dve(
                        ELU_SEL, out=helu, in0=hu, in1=eh,
                        s0=1.0, s1=0.0, imm2=0.0,
                    )
                    pT = pss.tile([128, 128], dt, tag="tr")
                    nc.tensor.transpose(out=pT, in_=helu, identity=ident_sb)
                    nc.scalar.copy(
                        out=hloc_sb[:, h * R + i4 * 128: h * R + (i4 + 1) * 128],
                        in_=pT)

            if debug_dump:
                nc.sync.dma_start(out=dbg_s1b, in_=s1b_sb)
                nc.sync.dma_start(out=dbg_smin, in_=smin_sb)
                nc.sync.dma_start(out=dbg_hloc, in_=hloc_sb)
                nc.sync.dma_start(out=dbg_wh, in_=wh_sb)

            if phases == "l1":
                for rt in range(RT):
                    nc.sync.dma_start(out=out_d[rt * 128:(rt + 1) * 128, :],
                                      in_=identf_sb[:, 0:F2])
                continue

            # ---- layer-2 local projections + gather payload ---------------
            # w2a cols per ct2: [W2(64) | w2p | w2m | pad]; pW = [Wh2 | s+ | s-]
            sp2_sb = work.tile([128, RT], dt, tag="sp2")
            gs_sb = const.tile([128, RT * PAY], dt)
            for rt in range(RT):
                pW = pss.tile([128, PAY], f32, tag="pw", bufs=2)
                for ct in range(CT2):
                    nc.tensor.matmul(
                        out=pW,
                        lhsT=hloc_sb[:, ct * R + rt * 128: ct * R + (rt + 1) * 128],
                        rhs=w2a_sb[:, ct * PAY: (ct + 1) * PAY],
                        start=(ct == 0), stop=(ct == CT2 - 1),
                    )
                gs = gs_sb[:, rt * PAY: (rt + 1) * PAY]
                nc.scalar.copy(out=gs[:, 0:F2], in_=pW[:, 0:F2])
                nc.vector.memset(gs[:, F2: F2 + 1], 1.0)
                nc.vector.tensor_copy(
                    out=gs[:, F2 + 1: F2 + 2], in_=pW[:, F2 + 1: F2 + 2])
                nc.vector.tensor_copy(
                    out=sp2_sb[:, rt: rt + 1], in_=pW[:, F2: F2 + 1])
            nc.sync.dma_start(out=gsend_t, in_=gs_sb)

            # local s+ -> s1b2 broadcast (no gather dependency): transpose
            # each sp2 column to a partition-0 row, then one ones-matmul.
            s1r2f_sb = const.tile([1, R], dt)
            for rt in range(RT):
                pt1 = pss.tile([1, 128], dt, tag="tr")
                nc.tensor.transpose(
                    out=pt1, in_=sp2_sb[:, rt: rt + 1], identity=ident_sb)
                nc.vector.tensor_copy(
                    out=s1r2f_sb[0:1, rt * 128: (rt + 1) * 128], in_=pt1)
            pbc2 = pss.tile([128, R], f32, tag="tr")
            nc.tensor.matmul(out=pbc2, lhsT=ones_row, rhs=s1r2f_sb,
                             start=True, stop=True)
            s1b2_sb = const.tile([128, R], dt)
            nc.scalar.copy(out=s1b2_sb, in_=pbc2)

            if collective:
                nc.gpsimd.collective_compute(
                    "AllGather", bypass,
                    replica_groups=[list(range(cfg.CORES))],
                    ins=[gsend_t.opt()], outs=[gfull_t.opt()],
                )
            else:
                nc.gpsimd.dma_start(
                    out=gfull_t[0:128, :], in_=gsend_t[:, :])

            gf_sb = const.tile([128, JT * PAY], dt)
            gff_v = gfull_t.rearrange("(c p) x -> p c x", p=128)
            _gfq = [nc.sync, nc.sync, nc.sync, nc.sync]
            for cc in range(4):
                _gfq[cc].dma_start(
                    out=gf_sb[:, cc * 2 * RT * PAY: (cc + 1) * 2 * RT * PAY]
                    .rearrange("p (c x) -> p c x", c=2),
                    in_=gff_v[:, cc * 2: cc * 2 + 2, :],
                )
            s2pf = const.tile([128, JT], f32)
            nc.vector.tensor_copy(
                out=s2pf[:, :].rearrange("p (t o) -> p t o", o=1),
                in_=gf_sb.rearrange("p (t q) -> p t q", q=PAY)[
                    :, :, F2 + 1: F2 + 2],
            )

            # ---- layer-2 attention + aggregation --------------------------
            psum2 = ps2.tile([F2 + 1, R], f32)
            for g in range(NG):
                ug = wz.tile([128, GRP * R], dt, tag="ug")
                zg = wz.tile([128, GRP * R], dt, tag="zg")
                for k in range(GRP):
                    t = g * GRP + k
                    nc.vector._custom_dve(
                        SCORE_LRELU,
                        out=ug[:, k * R: (k + 1) * R],
                        in0=s1b2_sb,
                        in1=mb_sb[:, t * R: (t + 1) * R],
                        s0=s2pf[:, t: t + 1], s1=0.0, imm2=ALPHA,
                    )
                half = GRP * R // 2
                nc.scalar.activation(out=zg[:, 0:half], in_=ug[:, 0:half],
                                     func=Exp)
                nc.scalar.activation(out=zg[:, half:], in_=ug[:, half:],
                                     func=Exp)
                for k in range(GRP):
                    t = g * GRP + k
                    nc.tensor.matmul(
                        out=psum2,
                        lhsT=gf_sb[:, t * PAY: t * PAY + F2 + 1],
                        rhs=zg[:, k * R: (k + 1) * R],
                        start=(t == 0), stop=(t == JT - 1),
                    )

            # ---- finalize: transpose, normalize, store --------------------
            o2 = const.tile([F2 + 1, R], f32)
            nc.scalar.copy(out=o2, in_=psum2)
            of_sb = const.tile([128, RT * F2], f32)
            for rt in range(RT):
                pT2 = pss.tile([128, F2 + 1], f32, tag="pw", bufs=2)
                nc.tensor.transpose(
                    out=pT2,
                    in_=o2[:, rt * 128: (rt + 1) * 128],
                    identity=identf_sb[0: F2 + 1, 0: F2 + 1],
                )
                rc = work.tile([128, 1], f32, tag="rc")
                nc.vector.reciprocal(out=rc, in_=pT2[:, F2: F2 + 1])
                nc.vector.tensor_scalar(
                    out=of_sb[:, rt * F2: (rt + 1) * F2],
                    in0=pT2[:, 0:F2], scalar1=rc, scalar2=0.0,
                    op0=mult, op1=bypass,
                )
            nc.sync.dma_start(
                out=out_d.rearrange("(rt p) f -> p rt f", p=128),
                in_=of_sb.rearrange("p (rt f) -> p rt f", rt=RT),
            )

    nc.compile()
    return nc


# --------------------------------------------------------------------------
# Host-side prep / sharding
# --------------------------------------------------------------------------

def host_prep(cfg: Cfg, g, inputs, W1, a1, W2, a2):
    N, C, H, HEADS, F2, R = cfg.N, cfg.C, cfg.H, cfg.HEADS, cfg.F2, cfg.R
    CT, CT2, PW1, PAY = cfg.CT, cfg.CT2, cfg.PW1, cfg.PAY
    X = np.asarray(inputs, np.float32)
    W1 = np.asarray(W1, np.float32)
    a1 = np.asarray(a1, np.float32)
    W2 = np.asarray(W2, np.float32)
    a2 = np.asarray(a2, np.float32)

    def tile128(A):
        # [k*128, cols] row-major -> partition-major [128, k*cols]
        k = A.shape[0] // 128
        return np.ascontiguousarray(
            A.reshape(k, 128, A.shape[1]).transpose(1, 0, 2).reshape(128, -1)
        )

    XT = np.ascontiguousarray(X.T)                                    # [C, N]

    # w1cs: per ct block [128, PW1]: [W1_h0 |0| W1_h1 |0| W1_h2 |0| W1_h3 |0|
    #                                 ws1m_0..3]
    ws1p = np.stack([W1[h] @ a1[h, :H, 0] for h in range(HEADS)], axis=1)  # [C,4]
    ws1m = np.stack([W1[h] @ a1[h, H:, 0] for h in range(HEADS)], axis=1)  # [C,4]
    w1cs = np.zeros((C, PW1), np.float32)
    for h in range(HEADS):
        w1cs[:, h * 129: h * 129 + 128] = W1[h]
    w1cs[:, HEADS * 129:] = ws1m
    w1cs_t = tile128(w1cs.astype(BF16))
    wsp_t = tile128(ws1p.astype(BF16))

    # w2a: per ct2 block [128, PAY]: [W2(64) | w2p | w2m | pad]
    w2p = W2 @ a2[:F2, 0]                                             # [HH]
    w2m = W2 @ a2[F2:, 0]                                             # [HH]
    w2a = np.zeros((HEADS * H, PAY), np.float32)
    w2a[:, :F2] = W2
    w2a[:, F2] = w2p
    w2a[:, F2 + 1] = w2m
    w2a_t = tile128(w2a.astype(BF16))

    ident = np.eye(128, dtype=BF16)
    identf = np.eye(128, dtype=np.float32)
    wpack = np.concatenate([w1cs_t, wsp_t, w2a_t, ident], axis=1)

    adj = np.asarray(g) > 0
    in_maps = []
    for c in range(cfg.CORES):
        rows = slice(c * R, (c + 1) * R)
        mb = np.where(adj[rows].T, 0.0, MASKBIAS).astype(BF16)        # [N, R]
        in_maps.append({
            "xtloc": tile128(np.ascontiguousarray(XT[:, rows]).astype(BF16)),
            "mb": tile128(np.ascontiguousarray(mb)),
            "wpack": wpack, "identf": identf,
        })
    return in_maps


_NC_CACHE = {}


def get_compiled(cfg: Cfg):
    nc = _NC_CACHE.get(cfg)
    if nc is None:
        nc = build_gat_nc(cfg)
        _NC_CACHE[cfg] = nc
    return nc


def kernel(g, inputs, W1, a1, W2, a2):
    cfg = FULL
    nc = get_compiled(cfg)
    in_maps = host_prep(cfg, g, inputs, W1, a1, W2, a2)
    res = run_bass_kernel_spmd(nc, in_maps, core_ids=list(range(cfg.CORES)))
    out = np.concatenate(
        [np.asarray(res.results[c]["out"], np.float32) for c in range(cfg.CORES)],
        axis=0,
    )
    return out
